# revision 62
# baseline (speedup 1.0000x reference)
"""Bi-path windowed attention kernel for Trainium2 (8 NeuronCores), v2.

Problem: x (4, 512, 128, 128) f32. Reference (per batch): raw-reshape to
tokens (128,128,512); global path = 2x2-window MHA (8 heads, hd=64) +
out-proj; local path = AvgPool2(x) -> raw-reshape tokens (64,64,512) ->
2x2-window MHA -> raw-reshape -> reflect-pad smoothing -> bilinear x2
upsample; out = (global + local) transposed to (B, C, H, W).

Sharding: 8 shards = batch (4) x channel-half (2); channel half h of x ==
token rows [64h, 64h+64), and both paths for those rows stay inside the
shard, so shards are independent.

v2 design (vs v1 elementwise attention): tokens-on-partitions, all-bf16
matmul pipeline with the attention itself on the PE:
 - tile = 32 windows x 4 tokens = 128 partitions (p = di*64 + 2w + dj)
 - Q^T/K^T computed directly in [head-dim, token] layout (lhsT = Wq/Wk
   chunks, rhs = x^T chunks); V token-major.
 - scores: per head one [64]-contraction matmul giving all 128x128 token
   pairs of the 32-window group, plus a rank-32 accumulation matmul that
   adds +C to same-window pairs (uniform boost cancels in softmax; the
   off-window pairs stay ~exp(-C/8) smaller = masked).
 - softmax without max-subtraction; q-bias cancels in softmax, k-bias
   becomes a per-token factor ev = exp(scale*K~_h.bq_h) folded into V and
   into a 65th "ones" column that makes the P.V matmul also emit the
   softmax denominator. v-bias: global path folds bv@Wproj+bproj into an
   output bias added via the smoothing stage; local path adds bv to V.
 - local pooling is 4 contiguous-token adds in token-major layout (the
   raw reshape makes pooled-token gathers contiguous in DRAM).
 - layout exchanges (token-major <-> channel-major) are free flat
   reinterpretations of DRAM scratch.
"""
import sys
if '/opt/trn_rl_repo' not in sys.path:
    sys.path.append('/opt/trn_rl_repo')
import numpy as np

_RUN_CACHE = {}

B, C, H, W = 4, 512, 128, 128
NH, HD = 8, 64
SCALE = float(HD) ** -0.5
MSQ = 11.3125   # bf16(sqrt(128)); MSQ^2 ~ 128 uniform in-window boost


def _mk_tile_context_fixed():
    import concourse.mybir as mybir
    import concourse.tile as tile
    from concourse.vector_clock import ScopedClock, VectorClock

    class TileContextFixed(tile.TileContext):
        """Works around a walrus codegen limit in this toolchain: max ONE
        sync-wait per instruction. Extra waits are peeled onto single-wait
        NoOps on the same engine; the kernel-tail drain gets per-proc
        single-wait NOPs instead of one multi-wait drain."""
        _ctr = 0

        def _lower_ordered_insts(self, ordered):
            cls = type(self)
            for bb_name, insts in ordered.items():
                new_list = []
                for inst in insts:
                    try:
                        si = inst.sync_info
                    except Exception:
                        si = None
                    if si is not None and len(si.on_wait) > 1:
                        waits = list(si.on_wait)
                        extra, keep = waits[:-1], waits[-1:]
                        for w in extra:
                            nop = mybir.InstNoOp(
                                name=f"I-waitsplit-{cls._ctr}", ins=[], outs=[])
                            cls._ctr += 1
                            nop.engine = inst.engine
                            nop.sync_info = mybir.SyncInfo(
                                on_wait=[w], on_update=[])
                            self.nc.register_instruction(nop, overwrite=True)
                            new_list.append(nop)
                        inst.sync_info = mybir.SyncInfo(
                            on_wait=keep, on_update=list(si.on_update))
                    new_list.append(inst)
                ordered[bb_name] = new_list
            super()._lower_ordered_insts(ordered)

        def _drain_and_barrier(self, tick_clock, wait_clock):
            gc = tick_clock.global_clock
            scoped = gc if hasattr(gc, 'items') else ScopedClock({None: gc})
            for scope, vc in scoped.items():
                n = len(vc)
                for proc in range(n):
                    t = vc[proc]
                    if t <= 0:
                        continue
                    vec = [0] * n
                    vec[proc] = t
                    nop = self.nc.sync.nop()
                    wait_clock.add_sem_waits(
                        nop.ins, ScopedClock({scope: VectorClock(vec)}))
            self.nc.sync.drain()
            self.nc.all_engine_barrier()
            popped = self.nc._tile_sem_poison_stack.pop()
            assert popped is self._sem_poison
            self.nc.clear_and_free_semaphores(
                list(self.sems.allocated().values()))
            self.nc.all_engine_barrier()

    return TileContextFixed


def _dap(handle, off, dims):
    """Raw DRAM access pattern: flat element offset + [step, count] dims."""
    import concourse.bass as bass
    base = handle[:]
    return bass.AP(tensor=base.tensor, offset=base.offset + off,
                   ap=[list(d) for d in dims])


def _sap(tile_, off, dims):
    """SBUF tile sub-AP: keep partition dim, replace free dims."""
    import concourse.bass as bass
    base = tile_[:]
    return bass.AP(tensor=base.tensor, offset=base.offset + off,
                   ap=[list(base.ap[0])] + [list(d) for d in dims])


def _build_nc():
    import concourse.bass as bass
    import concourse.mybir as mybir
    from concourse.masks import make_identity
    TileContextFixed = _mk_tile_context_fixed()
    f32 = mybir.dt.float32
    bf = mybir.dt.bfloat16
    Copy = mybir.ActivationFunctionType.Copy
    Exp = mybir.ActivationFunctionType.Exp
    ADD = mybir.AluOpType.add
    MUL = mybir.AluOpType.mult
    THIRD = 1.0 / 3.0

    nc = bass.Bass()
    xs = nc.declare_dram_parameter("xs", [8192 * 512], bf, isOutput=False)
    wg_d = nc.declare_dram_parameter("wqkv_g", [C * 3 * C], bf, isOutput=False)
    wl_d = nc.declare_dram_parameter("wqkv_l", [C * 3 * C], bf, isOutput=False)
    wp_d = nc.declare_dram_parameter("wproj", [C * C], bf, isOutput=False)
    bq_d = nc.declare_dram_parameter("bqmat", [C * 8], bf, isOutput=False)
    mb_d = nc.declare_dram_parameter("maskb", [32 * 128], bf, isOutput=False)
    ob_d = nc.declare_dram_parameter("obias", [C], bf, isOutput=False)
    bv_d = nc.declare_dram_parameter("bvv", [C], bf, isOutput=False)
    out = nc.declare_dram_parameter("out", [C, 64, W], f32, isOutput=True)

    lout = nc.dram_tensor("lout", [2048 * 512], bf)
    lup = nc.dram_tensor("lup", [8192 * 512], bf)

    with TileContextFixed(nc) as tc:
        with (
            tc.tile_pool(name="consts", bufs=1) as consts,
            tc.tile_pool(name="work", bufs=2) as work,
            tc.tile_pool(name="work3", bufs=6) as work3,
            tc.tile_pool(name="ps2", bufs=2, space="PSUM") as ps2,
            tc.tile_pool(name="ps1", bufs=1, space="PSUM") as ps1,
        ):
            # ---- constants ----
            identb = consts.tile([128, 128], bf)
            make_identity(nc, identb[:])
            identf = consts.tile([128, 128], f32)
            make_identity(nc, identf[:])
            wl = consts.tile([128, 4, 1536], bf)
            nc.sync.dma_start(out=wl, in_=_dap(
                wl_d, 0, [[1536, 128], [196608, 4], [1, 1536]]))
            wg = consts.tile([128, 4, 1536], bf)
            wpj = consts.tile([128, 4, 512], bf)
            bqb = consts.tile([128, 4, 8], bf)
            nc.sync.dma_start(out=bqb, in_=_dap(
                bq_d, 0, [[8, 128], [1024, 4], [1, 8]]))
            mbt = consts.tile([32, 128], bf)
            nc.sync.dma_start(out=mbt, in_=_dap(mb_d, 0, [[128, 32], [1, 128]]))
            obb = consts.tile([128, 512], bf)
            nc.sync.dma_start(out=obb, in_=_dap(ob_d, 0, [[0, 128], [1, 512]]))
            bvb = consts.tile([128, 512], bf)
            nc.sync.dma_start(out=bvb, in_=_dap(bv_d, 0, [[0, 128], [1, 512]]))

            # ---- stage emitters (software-pipelined across tiles) ----
            # xt: [128 c-in-chunk, 4 chunk, 128 tok] bf16 = x^T for a tile
            # of 128 tokens (p = di*64 + 2w + dj).
            def stage_qkv(xt, wt, add_bv, kt_on_dve=False):
                # QT/KT: [hd, tok] per 128-chunk of (h,d)
                psQT = ps2.tile([128, 512], f32, tag="mm")
                for hc in range(4):
                    for kc in range(4):
                        nc.tensor.matmul(
                            psQT[:, hc * 128:(hc + 1) * 128],
                            wt[:, kc, hc * 128:(hc + 1) * 128],
                            xt[:, kc, :], start=(kc == 0), stop=(kc == 3))
                qt = work3.tile([128, 4, 128], bf, tag="qt")
                nc.scalar.copy(out=_sap(qt, 0, [[1, 512]]), in_=psQT[:])
                psKT = ps2.tile([128, 512], f32, tag="mm")
                for hc in range(4):
                    for kc in range(4):
                        nc.tensor.matmul(
                            psKT[:, hc * 128:(hc + 1) * 128],
                            wt[:, kc, 512 + hc * 128:512 + (hc + 1) * 128],
                            xt[:, kc, :], start=(kc == 0), stop=(kc == 3))
                kt = work3.tile([128, 4, 128], bf, tag="kt")
                if kt_on_dve:
                    nc.vector.tensor_copy(_sap(kt, 0, [[1, 512]]), psKT[:])
                else:
                    nc.scalar.copy(out=_sap(kt, 0, [[1, 512]]), in_=psKT[:])
                # kb[tok, h] = K~_h . bq_h  (psum bank shared with finT)
                psKB = ps1.tile([128, 512], f32, tag="kbtrf")
                for hc in range(4):
                    nc.tensor.matmul(psKB[:, 0:8], kt[:, hc, :],
                                     bqb[:, hc, :],
                                     start=(hc == 0), stop=(hc == 3))
                ev = work.tile([128, 8], f32, tag="ev")
                nc.scalar.activation(ev[:], psKB[:, 0:8], Exp, scale=SCALE)
                # V (after QT/KT so the "mm" rotation can't deadlock)
                psV = ps2.tile([128, 512], f32, tag="mm")
                for kc in range(4):
                    nc.tensor.matmul(psV[:], xt[:, kc, :],
                                     wt[:, kc, 1024:1536],
                                     start=(kc == 0), stop=(kc == 3))
                vs = work3.tile([128, 8, 65], bf, tag="vs")
                if add_bv:
                    vb = work.tile([128, 512], f32, tag="vb")
                    nc.vector.tensor_add(vb[:], psV[:], bvb[:])
                    nc.vector.tensor_mul(
                        _sap(vs, 0, [[65, 8], [1, 64]]),
                        _sap(vb, 0, [[64, 8], [1, 64]]),
                        _sap(ev, 0, [[1, 8], [0, 64]]))
                else:
                    nc.vector.tensor_mul(
                        _sap(vs, 0, [[65, 8], [1, 64]]),
                        _sap(psV, 0, [[64, 8], [1, 64]]),
                        _sap(ev, 0, [[1, 8], [0, 64]]))
                nc.scalar.copy(out=_sap(vs, 64, [[65, 8]]), in_=ev[:])
                return qt, kt, vs

            def stage_scores(qkv):
                qt, kt, vs = qkv
                # scores for both 4-head groups (mask accs grouped so their
                # stationary reloads back-to-back on HW)
                psSts = []
                for g in range(2):
                    psSt = ps2.tile([128, 512], f32, tag="st")
                    for hi in range(4):
                        h = g * 4 + hi
                        hc, hh = h // 2, (h % 2) * 64
                        nc.tensor.matmul(
                            psSt[:, hi * 128:(hi + 1) * 128],
                            kt[hh:hh + 64, hc, :], qt[hh:hh + 64, hc, :],
                            start=True, stop=False)
                        nc.tensor.matmul(
                            psSt[:, hi * 128:(hi + 1) * 128],
                            mbt[:], mbt[:], start=False, stop=True)
                    psSts.append(psSt)
                return psSts

            def stage_pv(qkv, psSts, oscale=None):
                qt, kt, vs = qkv
                ems = []
                for g in range(2):
                    em = work.tile([128, 4, 128], bf, tag="em")
                    nc.scalar.activation(_sap(em, 0, [[1, 512]]), psSts[g][:],
                                         Exp, scale=SCALE)
                    ems.append(em)
                O = work.tile([128, 512], bf, tag="ob")
                R = work.tile([128, 8], f32, tag="rc")
                psPVs = []
                for g in range(2):
                    psPV = ps2.tile([128, 512], f32, tag="pv")
                    for hi in range(4):
                        h = g * 4 + hi
                        nc.tensor.matmul(
                            psPV[:, hi * 65:hi * 65 + 65],
                            ems[g][:, hi, :], vs[:, h, :],
                            start=True, stop=True)
                    psPVs.append(psPV)
                for g in range(2):
                    nc.vector.reciprocal(
                        R[:, g * 4:(g + 1) * 4], _sap(psPVs[g], 64, [[65, 4]]))
                    if oscale is None:
                        nc.vector.tensor_mul(
                            _sap(O, g * 256, [[64, 4], [1, 64]]),
                            _sap(psPVs[g], 0, [[65, 4], [1, 64]]),
                            _sap(R, g * 4, [[1, 4], [0, 64]]))
                    else:
                        nc.vector.scalar_tensor_tensor(
                            out=_sap(O, g * 256, [[64, 4], [1, 64]]),
                            in0=_sap(psPVs[g], 0, [[65, 4], [1, 64]]),
                            scalar=oscale,
                            in1=_sap(R, g * 4, [[1, 4], [0, 64]]),
                            op0=MUL, op1=MUL)
                return O

            # ---- local path: 16 tiles, 1-tile software pipeline ----
            def local_load(I):
                praw = work.tile([128, 2048], bf, tag="pl")
                nc.sync.dma_start(out=praw, in_=_dap(
                    xs, I * 262144, [[2048, 128], [1, 2048]]))
                # pool taps: praw free = (k, e, ta, f*tb)
                p1 = work.tile([128, 1024], bf, tag="p1")
                nc.vector.tensor_add(
                    _sap(p1, 0, [[256, 4], [128, 2], [1, 128]]),
                    _sap(praw, 0, [[512, 4], [256, 2], [1, 128]]),
                    _sap(praw, 128, [[512, 4], [256, 2], [1, 128]]))
                xp = work.tile([128, 512], bf, tag="xg")
                nc.vector.tensor_add(
                    _sap(xp, 0, [[128, 4], [64, 2], [1, 64]]),
                    _sap(p1, 0, [[256, 4], [128, 2], [2, 64]]),
                    _sap(p1, 1, [[256, 4], [128, 2], [2, 64]]))
                psT = ps1.tile([128, 512], bf, tag="tr")
                for kc in range(4):
                    nc.tensor.transpose(
                        psT[:, kc * 128:(kc + 1) * 128],
                        xp[:, kc * 128:(kc + 1) * 128], identb[:])
                xt = work3.tile([128, 4, 128], bf, tag="xt")
                nc.scalar.copy(out=_sap(xt, 0, [[1, 512]]), in_=psT[:])
                return xt

            def local_out(I, O):
                nc.sync.dma_start(
                    out=_dap(lout, I * 65536,
                             [[32768, 2], [1024, 32], [512, 2], [1, 512]]),
                    in_=O)

            # ---- smoothing + bilinear x2 upsample (channel-major) ----
            # a1[y] = l[y-1]+l[y] (reflect y=0), a2[x] = l[x]+l[x+1]
            # (reflect x=63), sraw = a1+a2; u'[2y] = sraw[y]+sraw[y-1]/3,
            # u'[2y+1] = sraw[y]+sraw[y+1]/3 (clamped); same along x; then
            # lup = 0.28125*L' + obias (global-path out bias folded here).
            # Emitted as strips interleaved into the local/global pipelines.
            def smooth_strip(cc, st):
                    y0 = st * 16
                    r0, r1 = max(y0 - 2, 0), min(y0 + 17, 64)   # Lp rows
                    s0, s1 = max(y0 - 1, 0), min(y0 + 17, 64)   # sraw rows
                    nlr = r1 - r0
                    nsr = s1 - s0
                    Lp = work.tile([128, nlr * 64], bf, tag="lp")
                    nc.sync.dma_start(out=Lp, in_=_dap(
                        lout, cc * 128 * 4096 + r0 * 64,
                        [[4096, 128], [1, nlr * 64]]))

                    def lrow(y):
                        return (y - r0) * 64

                    def srow(y):
                        return (y - s0) * 64

                    a1 = work.tile([128, nsr * 64], bf, tag="a1")
                    ym = max(s0, 1)  # main region rows [ym, s1)
                    nc.vector.tensor_add(
                        _sap(a1, srow(ym), [[1, (s1 - ym) * 64]]),
                        _sap(Lp, lrow(ym - 1), [[1, (s1 - ym) * 64]]),
                        _sap(Lp, lrow(ym), [[1, (s1 - ym) * 64]]))
                    if s0 == 0:  # reflect top: a1[0] = l[0] + l[1]
                        nc.vector.tensor_add(
                            _sap(a1, 0, [[1, 64]]),
                            _sap(Lp, 0, [[1, 64]]),
                            _sap(Lp, 64, [[1, 64]]))
                    a2 = work.tile([128, nsr * 64], bf, tag="a2")
                    nc.gpsimd.tensor_add(
                        _sap(a2, 0, [[64, nsr], [1, 63]]),
                        _sap(Lp, lrow(s0), [[64, nsr], [1, 63]]),
                        _sap(Lp, lrow(s0) + 1, [[64, nsr], [1, 63]]))
                    nc.gpsimd.tensor_add(
                        _sap(a2, 63, [[64, nsr]]),
                        _sap(Lp, lrow(s0) + 63, [[64, nsr]]),
                        _sap(Lp, lrow(s0) + 62, [[64, nsr]]))
                    sraw = work.tile([128, nsr * 64], bf, tag="sr")
                    nc.vector.tensor_add(sraw[:], a1[:], a2[:])
                    # sdiv = sraw/3 on Act so the upsample taps are plain
                    # 2x-mode tensor_adds on DVE
                    sdiv = work.tile([128, nsr * 64], bf, tag="sd")
                    nc.scalar.activation(sdiv[:], sraw[:], Copy, scale=THIRD)
                    # y-upsample (u' rows Y-2*y0, 32 rows x 64 cols)
                    u = work.tile([128, 2048], bf, tag="uu")
                    ye = max(y0, 1)  # even rows needing y-1
                    nc.vector.tensor_add(
                        _sap(u, (ye - y0) * 128,
                             [[128, y0 + 16 - ye], [1, 64]]),
                        _sap(sdiv, srow(ye - 1),
                             [[64, y0 + 16 - ye], [1, 64]]),
                        _sap(sraw, srow(ye), [[64, y0 + 16 - ye], [1, 64]]))
                    if y0 == 0:  # Y=0: taps both row 0
                        nc.vector.tensor_add(
                            _sap(u, 0, [[1, 64]]),
                            _sap(sdiv, 0, [[1, 64]]),
                            _sap(sraw, 0, [[1, 64]]))
                    yo1 = min(y0 + 16, 63)  # odd rows needing y+1
                    nc.vector.tensor_add(
                        _sap(u, 64, [[128, yo1 - y0], [1, 64]]),
                        _sap(sdiv, srow(y0 + 1), [[64, yo1 - y0], [1, 64]]),
                        _sap(sraw, srow(y0), [[64, yo1 - y0], [1, 64]]))
                    if y0 + 16 == 64:  # Y=127: taps both row 63
                        nc.vector.tensor_add(
                            _sap(u, 31 * 64, [[1, 64]]),
                            _sap(sdiv, srow(63), [[1, 64]]),
                            _sap(sraw, srow(63), [[1, 64]]))
                    udiv = work.tile([128, 2048], bf, tag="ud")
                    nc.scalar.activation(udiv[:], u[:], Copy, scale=THIRD)
                    # x-upsample per 16-row half + obias (0.28125 is folded
                    # into the local-path division upstream)
                    for hf in range(2):
                        Lh = work.tile([128, 2048], bf, tag="lh")
                        ub = hf * 16 * 64
                        nc.vector.tensor_add(
                            _sap(Lh, 2, [[128, 16], [2, 63]]),
                            _sap(udiv, ub, [[64, 16], [1, 63]]),
                            _sap(u, ub + 1, [[64, 16], [1, 63]]))
                        nc.vector.tensor_add(
                            _sap(Lh, 0, [[128, 16]]),
                            _sap(udiv, ub, [[64, 16]]),
                            _sap(u, ub, [[64, 16]]))
                        nc.vector.tensor_add(
                            _sap(Lh, 1, [[128, 16], [2, 63]]),
                            _sap(udiv, ub + 1, [[64, 16], [1, 63]]),
                            _sap(u, ub, [[64, 16], [1, 63]]))
                        nc.vector.tensor_add(
                            _sap(Lh, 127, [[128, 16]]),
                            _sap(udiv, ub + 63, [[64, 16]]),
                            _sap(u, ub + 63, [[64, 16]]))
                        Lsc = work.tile([128, 2048], bf, tag="ls")
                        eng = nc.gpsimd if hf == 0 else nc.vector
                        eng.tensor_add(
                            _sap(Lsc, 0, [[512, 4], [128, 4], [1, 128]]),
                            _sap(Lh, 0, [[512, 4], [128, 4], [1, 128]]),
                            _sap(obb, 0, [[0, 4], [128, 4], [1, 128]]))
                        nc.sync.dma_start(
                            out=_dap(lup,
                                     cc * 128 * 16384
                                     + (2 * y0 + 16 * hf) * 128,
                                     [[16384, 128], [1, 2048]]),
                            in_=Lsc)

            # ---- global path: 64 tiles (I in [0,32), Jh in {0,1}),
            # software-pipelined: x-prefetch lag 2, qkv lag 1; cc1
            # smoothing strips interleave into the first tiles ----
            def gx_load(ti):
                I, Jh = ti // 2, ti % 2
                toff = (I * 256 + Jh * 64) * 512
                xt = work3.tile([128, 4, 128], bf, tag="xt")
                for di in range(2):
                    nc.sync.dma_start_transpose(
                        out=xt[:, :, di * 64:(di + 1) * 64],
                        in_=_dap(xs, toff + di * 65536,
                                 [[512, 64], [1, 512]]))
                return xt

            def lup_load(ti):
                I, Jh = ti // 2, ti % 2
                toff = (I * 256 + Jh * 64) * 512
                lupt = work.tile([128, 512], bf, tag="lu")
                nc.sync.dma_start(out=lupt, in_=_dap(
                    lup, toff,
                    [[65536, 2], [1024, 32], [512, 2], [1, 512]]))
                return lupt

            def g_proj(O, lupt):
                psOt = ps1.tile([128, 512], bf, tag="tr")
                for kc in range(4):
                    nc.tensor.transpose(
                        psOt[:, kc * 128:(kc + 1) * 128],
                        O[:, kc * 128:(kc + 1) * 128], identb[:])
                ot = work.tile([128, 4, 128], bf, tag="ot")
                nc.vector.tensor_copy(_sap(ot, 0, [[1, 512]]), psOt[:])
                psP = ps2.tile([128, 512], f32, tag="mm")
                for kc in range(4):
                    nc.tensor.matmul(psP[:], ot[:, kc, :], wpj[:, kc, :],
                                     start=(kc == 0), stop=(kc == 3))
                t1 = work.tile([128, 512], bf, tag="t1")
                nc.vector.tensor_add(t1[:], psP[:], lupt[:])
                return t1

            def g_fin(ti, t1):
                I, Jh = ti // 2, ti % 2
                psF = ps1.tile([128, 512], bf, tag="tr")
                for kc in range(4):
                    nc.tensor.transpose(
                        psF[:, kc * 128:(kc + 1) * 128],
                        t1[:, kc * 128:(kc + 1) * 128], identb[:])
                fin = work.tile([128, 4, 128], f32, tag="fin")
                nc.vector.tensor_copy(_sap(fin, 0, [[1, 512]]), psF[:])
                for di in range(2):
                    nc.sync.dma_start(
                        out=_dap(out, I * 256 + di * 128 + Jh * 64,
                                 [[8192, 128], [1048576, 4], [1, 64]]),
                        in_=_sap(fin, di * 64, [[128, 4], [1, 64]]))

            # ---- local path: 16 tiles, load lag 3 / qkv lag 2; cc0
            # smoothing strips interleave once lout tiles 0-7 are done;
            # the global pipeline is warmed up in the tail iterations.
            gx, lu, gq = {}, {}, {}
            lx = {k: local_load(k) for k in range(3)}
            lq = {k: stage_qkv(lx.pop(k), wl, True) for k in range(2)}
            # global-path weights load behind the local prologue
            nc.sync.dma_start(out=wg, in_=_dap(
                wg_d, 0, [[1536, 128], [196608, 4], [1, 1536]]))
            nc.sync.dma_start(out=wpj, in_=_dap(
                wp_d, 0, [[512, 128], [65536, 4], [1, 512]]))
            for I in range(16):
                if I + 3 < 16:
                    lx[I + 3] = local_load(I + 3)
                psSts = stage_scores(lq[I])
                if I + 2 < 16:
                    lq[I + 2] = stage_qkv(lx.pop(I + 2), wl, True)
                local_out(I, stage_pv(lq.pop(I), psSts, oscale=0.28125))
                if I in (8, 9, 10, 11):
                    smooth_strip(0, I - 8)
                if I == 12:
                    gx[0] = gx_load(0)
                    gx[1] = gx_load(1)
                elif I == 13:
                    lu[0] = lup_load(0)
                    gq[0] = stage_qkv(gx.pop(0), wg, False)
                elif I == 14:
                    gx[2] = gx_load(2)
                    gq[1] = stage_qkv(gx.pop(1), wg, False)
                elif I == 15:
                    gx[3] = gx_load(3)
                    lu[1] = lup_load(1)

            fins = {}
            for ti in range(64):
                if ti + 2 < 64 and ti + 2 not in gx:
                    gx[ti + 2] = gx_load(ti + 2)
                if ti + 1 < 64 and ti + 1 not in lu:
                    lu[ti + 1] = lup_load(ti + 1)
                psSts = stage_scores(gq[ti])
                if ti + 1 < 64 and ti + 1 not in gq:
                    gq[ti + 1] = stage_qkv(gx.pop(ti + 1), wg, False)
                if ti - 1 in fins:
                    g_fin(ti - 1, fins.pop(ti - 1))
                O = stage_pv(gq.pop(ti), psSts)
                fins[ti] = g_proj(O, lu.pop(ti))
                if ti in (2, 8, 14, 20):
                    smooth_strip(1, (ti - 2) // 6)
            g_fin(63, fins.pop(63))
    return nc


def _get_nc():
    if 'nc' not in _RUN_CACHE:
        _RUN_CACHE['nc'] = _build_nc()
    return _RUN_CACHE['nc']


def make_in_maps(inputs):
    import ml_dtypes
    bf16 = ml_dtypes.bfloat16
    x = np.asarray(inputs['x'], dtype=np.float32)
    Wqkv = np.asarray(inputs['Wqkv'], dtype=np.float32)
    bqkv = np.asarray(inputs['bqkv'], dtype=np.float32)
    Wproj = np.asarray(inputs['Wproj'], dtype=np.float32)
    bproj = np.asarray(inputs['bproj'], dtype=np.float32)

    bq, bv = bqkv[:512], bqkv[1024:]
    wqkv_g = np.ascontiguousarray(Wqkv.astype(bf16)).reshape(-1)
    wqkv_l = np.ascontiguousarray((0.25 * Wqkv).astype(bf16)).reshape(-1)
    wproj = np.ascontiguousarray(Wproj.astype(bf16)).reshape(-1)
    bqmat = np.zeros((512, 8), np.float32)
    bqmat[np.arange(512), np.arange(512) // 64] = bq
    bqmat = bqmat.astype(bf16).reshape(-1)
    p = np.arange(128)
    maskb = np.zeros((32, 128), np.float32)
    maskb[(p % 64) // 2, p] = MSQ
    maskb = maskb.astype(bf16).reshape(-1)
    obias = (bproj + bv @ Wproj).astype(bf16)
    bvv = bv.astype(bf16)

    in_maps = []
    shards = []
    for b in range(B):
        for half in range(2):
            shards.append((b, half))
            xs = np.ascontiguousarray(
                x[b, 256 * half:256 * (half + 1)]).reshape(-1).astype(bf16)
            in_maps.append({
                "xs": xs, "wqkv_g": wqkv_g, "wqkv_l": wqkv_l,
                "wproj": wproj, "bqmat": bqmat, "maskb": maskb,
                "obias": obias, "bvv": bvv,
            })
    return in_maps, shards


def kernel(**inputs):
    from concourse.bass_utils import run_bass_kernel_spmd
    nc = _get_nc()
    in_maps, shards = make_in_maps(inputs)
    r = run_bass_kernel_spmd(nc, in_maps, core_ids=list(range(8)))
    _RUN_CACHE['last_result'] = r
    full = np.empty((B, C, H, W), dtype=np.float32)
    for (b, half), res in zip(shards, r.results):
        full[b, :, 64 * half:64 * (half + 1), :] = res["out"]
    return full
